# revision 35
# baseline (speedup 1.0000x reference)
"""DistMult decoder kernel for 8 Trainium2 NeuronCores.

Computes out = (input1 * weight[type_index]) @ input2.T + bias with
input1 [8192, 512], input2 [8192, 512] in fp32, out [8192, 8192].

Sharding: rows of input1 (and thus rows of the output) are split across
the 8 cores; input2 / weight / bias are replicated. No communication.

Per-core device program (M = 1024 rows):
  - lhsT  [512, 1024]  = w_r-scaled shard of input1, transposed + cast
    to fp16 on host (K-major)
  - rhs   [512, 8192]  = input2 transposed + cast to fp16 on host
  - fp16 operands run the PE at 1 col/cycle (4x fp32) with fp32 PSUM
    accumulation
  - output stored as fp16 (max |out| ~ 128, fp16 quantization adds
    ~5e-4 max-rel error; host upcasts to fp32) which halves the store
    traffic: 17 MB instead of 34 MB per core, taking DMA off the
    critical path (PE floor ~109 us, DMA now ~75 us)
  - loop structure: 16 halfgroups of 512 output columns; within one
    halfgroup, k-outer/m-inner over all 8 PSUM banks, so the first
    matmul only needs the first k-slice of rhs (128 KB) and of lhsT
    (256 KB) instead of the whole first column block
  - PSUM evacuation + bias add split between DVE (even m) and ACT
    (odd m), output stores alternate between the two HWDGE rings
"""

import os

import numpy as np

import concourse.bacc as bacc
import concourse.mybir as mybir
from concourse.bass_utils import run_bass_kernel_spmd
from concourse.tile import TileContext

N_CORES = 8
N1, N2, D = 8192, 8192, 512
M = N1 // N_CORES  # rows per core
P = 128            # partitions
KT = D // P        # 4 k-tiles
MT = M // P        # 8 m-tiles
HGW = 512          # halfgroup width = one psum bank of fp32
HG = N2 // HGW     # 16 halfgroups

# test.py hooks: set TRACE=True before calling kernel() to profile; the
# BassKernelResults of the last run lands in LAST_RESULTS.
TRACE = os.environ.get("BASS_KERNEL_TRACE", "0") == "1"
LAST_RESULTS = None

_cached_nc = None


def _build(bias_value):
    nc = bacc.Bacc(
        "TRN2", target_bir_lowering=False, debug=False, enable_asserts=False, num_devices=N_CORES
    )
    f32 = mybir.dt.float32
    f16 = mybir.dt.float16
    lhsT = nc.dram_tensor("lhsT", [D, M], f16, kind="ExternalInput")
    rhs = nc.dram_tensor("rhs", [D, N2], f16, kind="ExternalInput")
    out = nc.dram_tensor("out", [M, N2], f16, kind="ExternalOutput")

    # K-major DRAM views split into [P, KT, cols] for single-DMA loads.
    lhsT_r = lhsT[:, :].rearrange("(kt p) m -> p kt m", p=P)
    rhs_r = rhs[:, :].rearrange("(kt p) n -> p kt n", p=P)

    with TileContext(nc) as tc:
        with (
            tc.tile_pool(name="const", bufs=1) as constp,
            tc.tile_pool(name="lhs", bufs=1) as lhsp,
            tc.tile_pool(name="rhsp", bufs=2) as rhsp,
            tc.tile_pool(name="outp", bufs=12) as outp,
            tc.tile_pool(name="psum", bufs=4, space="PSUM") as psump,
        ):
            # Head loads. The binding constraint is the first matmul
            # (k=0, m=0): it needs only rhs k-slice 0 of halfgroup 0 and
            # lhsT k-slice 0, so those go first on the Sync HWDGE ring
            # (fastest first-byte). Each dma_start costs ~0.7 us of
            # issue time on its engine, so the rest is batched coarsely:
            # Sync carries the k=1..3 remainder of halfgroup 0, GpSimd
            # (SWDGE) carries the lhsT remainder and the halfgroup 1/2
            # prefetch. Scalar is blocked by the framework's
            # ACT_TABLE_LOAD until ~8.3 us, so it only gets the bias.
            lt = lhsp.tile([P, KT, M], f16, tag="lhs")
            rts = {}

            # Head loads. During the head the 16 SDMA engines round-
            # robin across all three DGE rings, so HBM bandwidth -- not
            # ring count -- is the scarce resource: everything the first
            # k-slices need goes out first, in per-k DMAs whose
            # completion semaphores fire incrementally (one aggregate
            # DMA would gate k=1 on the last byte of k=3). The two
            # first-matmul-critical slices (lhsT k0, rhs k0) lead the
            # two HWDGE rings in parallel; the rhs group-1 prefetch
            # rides SWDGE behind the tiny bias load, and later prefetch
            # is throttled by the rhs pool itself (bufs=2: the DMA for
            # group g+2 only issues once group g's last matmul retires,
            # so prefetch never competes with the critical head chain).
            rt0 = rhsp.tile([P, KT, HGW], f16, tag="rhs")
            rts[0] = rt0
            rt1 = rhsp.tile([P, KT, HGW], f16, tag="rhs", name="rt_1")
            rts[1] = rt1
            # Head DMAs in ~three tiers of ~128-256 KB chunks across
            # the three rings. Measured behavior: data drains at ~300+
            # GB/s, but each DMA's completion semaphore trails its data
            # by ~1 us and queues BEHIND the receipts of every earlier
            # DMA on its ring (3rd-position sems fire several us after
            # 2nd-position ones regardless of size). So: few DMAs,
            # everything the k-outer loop needs through k=3 in the
            # first two tiers, group-1 rhs in tier 3. The bias is baked
            # into the evacuation ops as an immediate (no DMA at all).
            H2 = 2 * HGW
            nc.sync.dma_start(out=lt[:, 0, 0:HGW], in_=lhsT_r[:, 0, 0:HGW])
            nc.scalar.dma_start(out=rt0[:, 0, :], in_=rhs_r[:, 0, 0:HGW])
            nc.gpsimd.dma_start(out=lt[:, 0, HGW:M], in_=lhsT_r[:, 0, HGW:M])
            nc.sync.dma_start(out=lt[:, 1, :], in_=lhsT_r[:, 1, :])
            nc.scalar.dma_start(out=rt0[:, 1:3, :], in_=rhs_r[:, 1:3, 0:HGW])
            nc.gpsimd.dma_start(out=lt[:, 2, :], in_=lhsT_r[:, 2, :])
            nc.sync.dma_start(out=rt0[:, 3, :], in_=rhs_r[:, 3, 0:HGW])
            nc.scalar.dma_start(out=lt[:, 3, :], in_=lhsT_r[:, 3, :])
            nc.gpsimd.dma_start(out=rt1[:, 0:2, :], in_=rhs_r[:, 0:2, HGW:H2])
            nc.sync.dma_start(out=rt1[:, 2:4, :], in_=rhs_r[:, 2:4, HGW:H2])

            def load_rhs(g, eng):
                rt = rhsp.tile([P, KT, HGW], f16, tag="rhs", name=f"rt_{g}")
                eng.dma_start(out=rt[:], in_=rhs_r[:, :, g * HGW : (g + 1) * HGW])
                rts[g] = rt

            # Warm up the PE's HAM clock gate while the head loads are
            # in flight: ~2 us of small matmuls on zeroed SBUF so the PE
            # is busy from ~7.4 us (right after the framework preamble)
            # and flips to 2.4 GHz ~3.4 us later. Kept short so the real
            # matmuls don't queue behind it once their data lands.
            warm_w = constp.tile([P, P], f16, tag="warmw")
            warm_r = constp.tile([P, P], f16, tag="warmr")
            nc.vector.memset(warm_w[:], 0.0)
            nc.vector.memset(warm_r[:], 0.0)
            # Bias comes in as a compile-time constant: materialized by
            # an on-chip memset (no DMA, no semaphore slot on any ring).
            bias_t = constp.tile([P, 1], f32, tag="bias")
            nc.vector.memset(bias_t[:], bias_value)
            wps = psump.tile([P, 2 * HGW], f32, tag="ps", name="wps")
            NWARM = 24
            for i in range(NWARM):
                nc.tensor.matmul(
                    wps[:, 0:P], warm_w[:], warm_r[:],
                    start=(i == 0), stop=(i == NWARM - 1),
                )

            for g in range(HG):
                rt = rts.pop(g)
                # One halfgroup of lookahead: the pool slot for g+2
                # frees (and its DMA issues) when g's matmuls finish,
                # leaving the full g+1 window (~7 us) for a ~2 us load.
                if g + 2 <= HG - 1:
                    load_rhs(g + 2, nc.gpsimd)
                # k-outer over all 8 psum banks: each rhs k-slice is
                # streamed through the PE for all 8 m-tiles before the
                # next k-slice is needed, so the head only waits on the
                # first 128 KB chunk, and each bank's accumulation
                # group finishes at k=3 with 7 matmuls of slack for the
                # evacuation engines.
                # PSUM is managed as four 2-bank pair tiles; each
                # matmul still targets a single bank (one half of a
                # pair) but evacuation reads a whole pair in one
                # [P, 1024] DVE/ACT op -- ~40% less evacuation engine
                # time than per-bank ops (the ~120-170 cycle fixed cost
                # amortizes) and fewer, larger (256 KB) stores. The
                # m-visit order (0,1,2,3,6,7,4,5) makes pairs finish in
                # the order the next halfgroup reuses them, leaving
                # every pair >=0.5 us of evacuation slack. GpSimd gets
                # no stores: an engine with in-flight SWDGE work at
                # kernel end pays a multi-us queue drain at the exit
                # barrier.
                MORD = (0, 1, 2, 3, 6, 7, 4, 5)
                pps = [
                    psump.tile([P, 2 * HGW], f32, tag="ps", name=f"ps_{g}_{p}")
                    for p in range(4)
                ]
                for k in range(KT):
                    for m in MORD:
                        nc.tensor.matmul(
                            pps[m // 2][:, (m % 2) * HGW : (m % 2 + 1) * HGW],
                            lt[:, k, m * P : (m + 1) * P],
                            rt[:, k, :],
                            start=(k == 0), stop=(k == KT - 1),
                        )
                ots = {}
                for p, eng in ((0, "dve"), (1, "act"), (3, "act"), (2, "dve")):
                    ot = outp.tile(
                        [P, 2 * HGW], f16, tag="ot", name=f"ot_{g}_{p}"
                    )
                    if eng == "dve":
                        nc.vector.tensor_scalar_add(ot[:], pps[p][:], bias_value)
                    else:
                        nc.scalar.activation(
                            ot[:], pps[p][:],
                            mybir.ActivationFunctionType.Identity,
                            bias=bias_t[:, 0:1],
                        )
                    ots[p] = ot
                for p in (0, 1, 3, 2):
                    st = nc.sync if p in (0, 2) else nc.scalar
                    dst = out[
                        2 * p * P : 2 * (p + 1) * P, g * HGW : (g + 1) * HGW
                    ].rearrange("(b p) n -> p b n", p=P)
                    st.dma_start(
                        out=dst,
                        in_=ots[p][:].rearrange("p (b n) -> p b n", b=2),
                    )
    nc.compile()
    return nc


def kernel(input1, input2, weight, bias, type_index):
    global _cached_nc, LAST_RESULTS

    input1 = np.asarray(input1, dtype=np.float32)
    input2 = np.asarray(input2, dtype=np.float32)
    weight = np.asarray(weight, dtype=np.float32)
    bias = np.asarray(bias, dtype=np.float32).reshape(-1)
    w_r = weight[int(type_index)]  # [D]

    # Host-side prep: fold the w_r row-scale into input1, lay both GEMM
    # operands out K-major, cast to fp16 (device accumulates in fp32).
    scaled = input1 * w_r[None, :]  # [N1, D]
    rhsT = np.ascontiguousarray(input2.T.astype(np.float16))  # [D, N2]

    in_maps = []
    for c in range(N_CORES):
        shard = scaled[c * M : (c + 1) * M]  # [M, D]
        in_maps.append(
            {
                "lhsT": np.ascontiguousarray(shard.T.astype(np.float16)),
                "rhs": rhsT,
            }
        )

    # The scalar bias is baked into the compiled program as an
    # immediate; rebuild if a different bias value ever shows up.
    bias_value = float(bias[0])
    if _cached_nc is None or _cached_nc[0] != bias_value:
        _cached_nc = (bias_value, _build(bias_value))

    res = run_bass_kernel_spmd(
        _cached_nc[1], in_maps, core_ids=list(range(N_CORES)), trace=TRACE
    )
    LAST_RESULTS = res
    out = np.concatenate([res.results[c]["out"] for c in range(N_CORES)], axis=0)
    return out.astype(np.float32)


# revision 38
# speedup vs baseline: 1.0306x; 1.0306x over previous
"""DistMult decoder kernel for 8 Trainium2 NeuronCores.

Computes out = (input1 * weight[type_index]) @ input2.T + bias with
input1 [8192, 512], input2 [8192, 512] in fp32, out [8192, 8192].

Sharding: rows of input1 (and thus rows of the output) are split across
the 8 cores; input2 / weight / bias are replicated. No communication.

Per-core device program (M = 1024 rows):
  - lhsT  [512, 1024]  = w_r-scaled shard of input1, transposed + cast
    to fp16 on host (K-major)
  - rhs   [512, 8192]  = input2 transposed + cast to fp16 on host
  - fp16 operands run the PE at 1 col/cycle (4x fp32) with fp32 PSUM
    accumulation
  - output stored as fp16 (max |out| ~ 128, fp16 quantization adds
    ~5e-4 max-rel error; host upcasts to fp32) which halves the store
    traffic: 17 MB instead of 34 MB per core, taking DMA off the
    critical path (PE floor ~109 us, DMA now ~75 us)
  - loop structure: 16 halfgroups of 512 output columns; within one
    halfgroup, k-outer/m-inner over all 8 PSUM banks, so the first
    matmul only needs the first k-slice of rhs (128 KB) and of lhsT
    (256 KB) instead of the whole first column block
  - PSUM evacuation + bias add split between DVE (even m) and ACT
    (odd m), output stores alternate between the two HWDGE rings
"""

import os

import numpy as np

import concourse.bacc as bacc
import concourse.mybir as mybir
from concourse.bass_utils import run_bass_kernel_spmd
from concourse.tile import TileContext

N_CORES = 8
N1, N2, D = 8192, 8192, 512
M = N1 // N_CORES  # rows per core
P = 128            # partitions
KT = D // P        # 4 k-tiles
MT = M // P        # 8 m-tiles
HGW = 512          # halfgroup width = one psum bank of fp32
HG = N2 // HGW     # 16 halfgroups

# test.py hooks: set TRACE=True before calling kernel() to profile; the
# BassKernelResults of the last run lands in LAST_RESULTS.
TRACE = os.environ.get("BASS_KERNEL_TRACE", "0") == "1"
LAST_RESULTS = None

_cached_nc = None


def _build(bias_value):
    nc = bacc.Bacc(
        "TRN2", target_bir_lowering=False, debug=False, enable_asserts=False, num_devices=N_CORES
    )
    f32 = mybir.dt.float32
    f16 = mybir.dt.float16
    lhsT = nc.dram_tensor("lhsT", [D, M], f16, kind="ExternalInput")
    rhs = nc.dram_tensor("rhs", [D, N2], f16, kind="ExternalInput")
    out = nc.dram_tensor("out", [M, N2], f16, kind="ExternalOutput")

    # K-major DRAM views split into [P, KT, cols] for single-DMA loads.
    lhsT_r = lhsT[:, :].rearrange("(kt p) m -> p kt m", p=P)
    rhs_r = rhs[:, :].rearrange("(kt p) n -> p kt n", p=P)

    with TileContext(nc) as tc:
        with (
            tc.tile_pool(name="const", bufs=1) as constp,
            tc.tile_pool(name="lhs", bufs=1) as lhsp,
            tc.tile_pool(name="rhsp", bufs=2) as rhsp,
            tc.tile_pool(name="outp", bufs=24) as outp,
            tc.tile_pool(name="psum", bufs=8, space="PSUM") as psump,
        ):
            # Head loads. The binding constraint is the first matmul
            # (k=0, m=0): it needs only rhs k-slice 0 of halfgroup 0 and
            # lhsT k-slice 0, so those go first on the Sync HWDGE ring
            # (fastest first-byte). Each dma_start costs ~0.7 us of
            # issue time on its engine, so the rest is batched coarsely:
            # Sync carries the k=1..3 remainder of halfgroup 0, GpSimd
            # (SWDGE) carries the lhsT remainder and the halfgroup 1/2
            # prefetch. Scalar is blocked by the framework's
            # ACT_TABLE_LOAD until ~8.3 us, so it only gets the bias.
            lt = lhsp.tile([P, KT, M], f16, tag="lhs")
            rts = {}

            # Head loads. During the head the 16 SDMA engines round-
            # robin across all three DGE rings, so HBM bandwidth -- not
            # ring count -- is the scarce resource: everything the first
            # k-slices need goes out first, in per-k DMAs whose
            # completion semaphores fire incrementally (one aggregate
            # DMA would gate k=1 on the last byte of k=3). The two
            # first-matmul-critical slices (lhsT k0, rhs k0) lead the
            # two HWDGE rings in parallel; the rhs group-1 prefetch
            # rides SWDGE behind the tiny bias load, and later prefetch
            # is throttled by the rhs pool itself (bufs=2: the DMA for
            # group g+2 only issues once group g's last matmul retires,
            # so prefetch never competes with the critical head chain).
            rt0 = rhsp.tile([P, KT, HGW], f16, tag="rhs")
            rts[0] = rt0
            rt1 = rhsp.tile([P, KT, HGW], f16, tag="rhs", name="rt_1")
            rts[1] = rt1
            # Head DMAs in ~three tiers of ~128-256 KB chunks across
            # the three rings. Measured behavior: data drains at ~300+
            # GB/s, but each DMA's completion semaphore trails its data
            # by ~1 us and queues BEHIND the receipts of every earlier
            # DMA on its ring (3rd-position sems fire several us after
            # 2nd-position ones regardless of size). So: few DMAs,
            # everything the k-outer loop needs through k=3 in the
            # first two tiers, group-1 rhs in tier 3. The bias is baked
            # into the evacuation ops as an immediate (no DMA at all).
            H2 = 2 * HGW
            nc.sync.dma_start(out=lt[:, 0, 0:HGW], in_=lhsT_r[:, 0, 0:HGW])
            nc.scalar.dma_start(out=rt0[:, 0, :], in_=rhs_r[:, 0, 0:HGW])
            nc.gpsimd.dma_start(out=lt[:, 0, HGW:M], in_=lhsT_r[:, 0, HGW:M])
            nc.sync.dma_start(out=lt[:, 1, :], in_=lhsT_r[:, 1, :])
            nc.scalar.dma_start(out=rt0[:, 1:3, :], in_=rhs_r[:, 1:3, 0:HGW])
            nc.gpsimd.dma_start(out=lt[:, 2, :], in_=lhsT_r[:, 2, :])
            nc.sync.dma_start(out=rt0[:, 3, :], in_=rhs_r[:, 3, 0:HGW])
            nc.scalar.dma_start(out=lt[:, 3, :], in_=lhsT_r[:, 3, :])
            nc.gpsimd.dma_start(out=rt1[:, 0:2, :], in_=rhs_r[:, 0:2, HGW:H2])
            nc.sync.dma_start(out=rt1[:, 2:4, :], in_=rhs_r[:, 2:4, HGW:H2])

            def load_rhs(g, eng):
                rt = rhsp.tile([P, KT, HGW], f16, tag="rhs", name=f"rt_{g}")
                eng.dma_start(out=rt[:], in_=rhs_r[:, :, g * HGW : (g + 1) * HGW])
                rts[g] = rt

            # Warm up the PE's HAM clock gate while the head loads are
            # in flight: ~2 us of small matmuls on zeroed SBUF so the PE
            # is busy from ~7.4 us (right after the framework preamble)
            # and flips to 2.4 GHz ~3.4 us later. Kept short so the real
            # matmuls don't queue behind it once their data lands.
            warm_w = constp.tile([P, P], f16, tag="warmw")
            warm_r = constp.tile([P, P], f16, tag="warmr")
            nc.vector.memset(warm_w[:], 0.0)
            nc.vector.memset(warm_r[:], 0.0)
            # Bias comes in as a compile-time constant: materialized by
            # an on-chip memset (no DMA, no semaphore slot on any ring).
            bias_t = constp.tile([P, 1], f32, tag="bias")
            nc.vector.memset(bias_t[:], bias_value)
            wps = psump.tile([P, HGW], f32, tag="ps", name="wps")
            NWARM = 24
            for i in range(NWARM):
                nc.tensor.matmul(
                    wps[:, 0:P], warm_w[:], warm_r[:],
                    start=(i == 0), stop=(i == NWARM - 1),
                )

            for g in range(HG):
                rt = rts.pop(g)
                # One halfgroup of lookahead: the pool slot for g+2
                # frees (and its DMA issues) when g's matmuls finish,
                # leaving the full g+1 window (~7 us) for a ~2 us load.
                if g + 2 <= HG - 1:
                    load_rhs(g + 2, nc.gpsimd)
                # k-outer over all 8 psum banks: each rhs k-slice is
                # streamed through the PE for all 8 m-tiles before the
                # next k-slice is needed, so the head only waits on the
                # first 128 KB chunk, and each bank's accumulation
                # group finishes at k=3 with 7 matmuls of slack for the
                # evacuation engines.
                pss = [
                    psump.tile([P, HGW], f32, tag="ps", name=f"ps_{g}_{m}")
                    for m in range(MT)
                ]
                for k in range(KT):
                    for m in range(MT):
                        nc.tensor.matmul(
                            pss[m][:], lt[:, k, m * P : (m + 1) * P],
                            rt[:, k, :],
                            start=(k == 0), stop=(k == KT - 1),
                        )
                # Bias-add + fp32->fp16 cast on the way out of PSUM.
                # Evacuation alternates DVE/ACT (m=6/7 swapped so the
                # last-finishing bank lands on the less-backlogged DVE).
                # Stores ride the Sync ring except two on Scalar after
                # its ACT work; GpSimd gets none -- an engine with
                # in-flight SWDGE work at kernel end pays a multi-us
                # queue drain before the exit barrier.
                ots = {}
                for m in range(MT):
                    ot = outp.tile([P, HGW], f16, tag="ot", name=f"ot_{g}_{m}")
                    dve = m % 2 == 0 if m < 6 else m == 7
                    if dve:
                        nc.vector.tensor_scalar_add(ot[:], pss[m][:], bias_value)
                    else:
                        nc.scalar.activation(
                            ot[:], pss[m][:],
                            mybir.ActivationFunctionType.Identity,
                            bias=bias_t[:, 0:1],
                        )
                    ots[m] = ot
                for m in (0, 1, 2, 4, 7, 6, 3, 5):
                    st = nc.sync if m not in (3, 5) else nc.scalar
                    st.dma_start(
                        out=out[m * P : (m + 1) * P, g * HGW : (g + 1) * HGW],
                        in_=ots[m][:],
                    )
    nc.compile()
    return nc


def kernel(input1, input2, weight, bias, type_index):
    global _cached_nc, LAST_RESULTS

    input1 = np.asarray(input1, dtype=np.float32)
    input2 = np.asarray(input2, dtype=np.float32)
    weight = np.asarray(weight, dtype=np.float32)
    bias = np.asarray(bias, dtype=np.float32).reshape(-1)
    w_r = weight[int(type_index)]  # [D]

    # Host-side prep: fold the w_r row-scale into input1, lay both GEMM
    # operands out K-major, cast to fp16 (device accumulates in fp32).
    scaled = input1 * w_r[None, :]  # [N1, D]
    rhsT = np.ascontiguousarray(input2.T.astype(np.float16))  # [D, N2]

    in_maps = []
    for c in range(N_CORES):
        shard = scaled[c * M : (c + 1) * M]  # [M, D]
        in_maps.append(
            {
                "lhsT": np.ascontiguousarray(shard.T.astype(np.float16)),
                "rhs": rhsT,
            }
        )

    # The scalar bias is baked into the compiled program as an
    # immediate; rebuild if a different bias value ever shows up.
    bias_value = float(bias[0])
    if _cached_nc is None or _cached_nc[0] != bias_value:
        _cached_nc = (bias_value, _build(bias_value))

    res = run_bass_kernel_spmd(
        _cached_nc[1], in_maps, core_ids=list(range(N_CORES)), trace=TRACE
    )
    LAST_RESULTS = res
    out = np.concatenate([res.results[c]["out"] for c in range(N_CORES)], axis=0)
    return out.astype(np.float32)
